# revision 12
# baseline (speedup 1.0000x reference)
"""Trainium2 Bass kernel for nn_CGRU (spectral-norm linear -> GRU x16 -> per-step
BatchNorm), 8-way model-parallel over the hidden dimension.

Shapes (hardcoded): B=256, Z=512, H=2048, T=16, 8 cores.

v2: all weight algebra (spectral-norm sigma, lin/fc folds into the GRU input
weights, bias folds) is done on the host once per call; the device runs only
the z-dependent recurrence:
  step 1:   gates = W0 @ z.T            (W0 = w_ih @ fc_w / sigma, host-folded)
  steps 2+: gates = W_comb @ h.T        (W_comb = w_ih @ lin_w (+ w_hh), folded)
  per step: 8-core AllGather of the h shard (f16), proj/BN of the previous h
            ride the gather window.
"""
import os
import sys
import types
import contextlib
import ctypes

import numpy as np
import ml_dtypes

import concourse.bass as bass
import concourse.bacc as bacc
import concourse.mybir as mybir
import concourse.tile as tile
from concourse.bass import ts
from concourse.bass_utils import run_bass_kernel_spmd
from concourse.masks import make_identity

f32 = mybir.dt.float32
f32r = mybir.dt.float32r
bf16 = mybir.dt.bfloat16
fp16 = mybir.dt.float16
AF = mybir.ActivationFunctionType
OP = mybir.AluOpType

B, Z, H, T, NC = 256, 512, 2048, 16, 8
HS = H // NC          # 256 hidden units per core (2 chunks of 128)
GR = 3 * HS           # 768 gate rows per core (r,z,n)
FR = 4 * HS           # 1024 fused rows per core (rz fused, in, hn)
ZS = Z // NC          # 64 output features per core
KC = H // 128         # 16 contraction chunks
EPS = 1e-5

XDT = os.environ.get("BASS_XDT", "f16")   # f16 | bf16 | f8 (f8 fails tolerance)
USE_BF16 = XDT == "bf16"
USE_F8 = XDT == "f8"
SW = 16.0 if USE_F8 else 1.0              # host-side weight scale (fp8 range)
ISW = 1.0 / SW

# vecs column map ([128, 64] fp32 scratch of per-partition scalars)
# bh_s[768] -> BH..BH+5 ; brz_s[512] -> BRZ.. ; cin_s[256] -> CIN.. ;
# brz1_s[512] -> BRZ1.. ; c1n_s[256] -> C1N.. ; BH16: 16*b_hh n-part (f8)
BH, BRZ, CIN, BRZ1, C1N, BH16 = 0, 6, 10, 12, 16, 18

LAST_EXEC_NS = [None]
LAST_RESULTS = [None]


def _install_ntff_hook():
    """The agent image lacks antenv.axon_hooks; recreate it so
    run_bass_kernel_spmd(trace=True) can capture NTFF profiles via the
    libaxon_pjrt.so C ABI (same as trn_agent_boot)."""
    try:
        import antenv
    except ImportError:
        return
    if "antenv.axon_hooks" in sys.modules:
        return
    so_path = "/opt/axon/libaxon_pjrt.so"
    if not os.path.exists(so_path):
        return
    lib = ctypes.CDLL(so_path)
    if not hasattr(lib, "axon_start_nrt_profile"):
        return
    lib.axon_start_nrt_profile.argtypes = [ctypes.POINTER(ctypes.c_int64), ctypes.c_size_t]
    lib.axon_start_nrt_profile.restype = ctypes.c_int64
    lib.axon_stop_nrt_profile.argtypes = [ctypes.c_char_p]
    lib.axon_stop_nrt_profile.restype = ctypes.c_int64

    @contextlib.contextmanager
    def _hook(output_dir, device_ids):
        import jax

        jax.devices()
        if device_ids:
            ids = (ctypes.c_int64 * len(device_ids))(*device_ids)
            rc = lib.axon_start_nrt_profile(ids, len(device_ids))
        else:
            rc = lib.axon_start_nrt_profile(None, 0)
        if rc != 0:
            raise RuntimeError(f"axon_start_nrt_profile rc={rc}")
        try:
            yield
        finally:
            n = lib.axon_stop_nrt_profile(str(output_dir).encode())
            print(f"profile: {n} file(s) written to {output_dir}", file=sys.stderr)

    mod = types.ModuleType("antenv.axon_hooks")
    _state = {"hook": _hook}
    mod.set_axon_ntff_profile_hook = lambda h: _state.__setitem__("hook", h)
    mod.get_axon_ntff_profile_hook = lambda: _state["hook"]
    sys.modules["antenv.axon_hooks"] = mod
    antenv.axon_hooks = mod


def _emit_rsqrt(nc, out_ap, v_ap, magic_ap, scr):
    """out = 1/sqrt(v) via bit-trick seed + 3 Newton iterations (DVE only).
    scr: [P, 8] fp32 scratch tile AP (cols 0..5 used)."""
    i32 = mybir.dt.int32
    P = v_ap.shape[0]
    c = lambda k: scr[0:P, k:k + 1]
    nc.vector.tensor_scalar(c(0).bitcast(i32), v_ap.bitcast(i32), 1, None,
                            OP.arith_shift_right)
    nc.vector.tensor_tensor(c(1).bitcast(i32), magic_ap[0:P, :], c(0).bitcast(i32),
                            OP.subtract)                      # y0
    nc.vector.tensor_scalar(c(2), v_ap, 0.5, None, OP.mult)   # hv
    ycols = (1, 5, 1)
    for it in range(3):
        y = c(ycols[it])
        nc.vector.scalar_tensor_tensor(c(3), y, c(2), y, OP.mult, OP.mult)  # p = y*hv*y
        nc.vector.tensor_scalar(c(4), c(3), -1.0, 1.5, OP.mult, OP.add)     # q = 1.5 - p
        dst = out_ap if it == 2 else c(ycols[it + 1])
        nc.vector.tensor_tensor(dst, y, c(4), OP.mult)


def build_nc():
    DT = mybir.dt.float8e4 if USE_F8 else (bf16 if USE_BF16 else fp16)

    nc = bacc.Bacc("TRN2", target_bir_lowering=False, debug=False, num_devices=NC)

    # ---- I/O (all weights host-folded) ----
    zT_in = nc.dram_tensor("zT", [Z, B], DT, kind="ExternalInput")
    w0T_in = nc.dram_tensor("w0T_s", [Z, GR], DT, kind="ExternalInput")
    wcT_in = nc.dram_tensor("wcT_s", [H, FR], DT, kind="ExternalInput")
    linwT_in = nc.dram_tensor("lin_wT_s", [H, ZS], DT, kind="ExternalInput")
    linbs_in = nc.dram_tensor("lin_b_s", [ZS, 1], f32, kind="ExternalInput")
    bh_in = nc.dram_tensor("bh_s", [GR], f32, kind="ExternalInput")
    brz_in = nc.dram_tensor("brz_s", [4 * 128], f32, kind="ExternalInput")
    cin_in = nc.dram_tensor("cin_s", [2 * 128], f32, kind="ExternalInput")
    brz1_in = nc.dram_tensor("brz1_s", [4 * 128], f32, kind="ExternalInput")
    c1n_in = nc.dram_tensor("c1n_s", [2 * 128], f32, kind="ExternalInput")
    y_out = nc.dram_tensor("y_part", [T, B, ZS], f32, kind="ExternalOutput")

    # per-step collective bounce buffers (ring of NB, reused across steps)
    NB = int(os.environ.get("BASS_CC_BUFS", "4"))
    cc_in = [nc.dram_tensor(f"cc_in{t}", [HS, B], DT) for t in range(NB)]
    cc_out = [
        nc.dram_tensor(f"cc_out{t}", [H, B], DT, addr_space="Shared")
        for t in range(NB)
    ]
    cc_in = [cc_in[t % NB] for t in range(T)]
    cc_out = [cc_out[t % NB] for t in range(T)]
    ccw_in = nc.dram_tensor("ccw_in", [8, 16], f32)
    ccw_out = nc.dram_tensor("ccw_out", [64, 16], f32, addr_space="Shared")
    rg = [list(range(NC))]

    with tile.TileContext(nc) as tc:
        with tc.tile_pool(name="perm", bufs=1) as perm:
            # fire a tiny AllGather immediately: ncfw first-collective warm-up
            # then runs concurrently with the setup DMA/compute.
            nc.gpsimd.collective_compute("AllGather", OP.bypass, replica_groups=rg,
                                         ins=[ccw_in.ap().opt()],
                                         outs=[ccw_out.ap().opt()])
            # ---- persistent SBUF ----
            W_all = perm.tile([128, KC, FR], DT, name="W_all")
            w0T_sb = perm.tile([128, 4, GR], DT, name="w0T_sb")
            linwT_sb = perm.tile([128, KC, ZS], DT, name="linwT_sb")
            h_T = perm.tile([128, KC, B], DT, name="h_T")
            h_new = perm.tile([128, 2, B], f32, name="h_new")
            h_new_x = perm.tile([128, 2, B], DT, name="h_new_x")
            vecs = perm.tile([128, 24], f32, name="vecs")
            ident = perm.tile([128, 128], f32, name="ident")
            lb_sb = perm.tile([ZS, 1], f32, name="lb_sb")
            magic_sb = perm.tile([128, 1], mybir.dt.int32, name="magic_sb")
            zT_sb = perm.tile([128, 4, B], DT, name="zT_sb")
            # gate work tiles (single-buffered, reused every step)
            r_sb = perm.tile([128, 2, B], f32, name="r_sb")
            u_sb = perm.tile([128, 2, B], f32, name="u_sb")
            in_sb = perm.tile([128, 2, B], f32, name="in_sb")
            pre_sb = perm.tile([128, 2, B], f32, name="pre_sb")
            d_sb = perm.tile([128, 2, B], f32, name="d_sb")
            e_sb = perm.tile([128, 2, B], f32, name="e_sb")

            sync = nc.sync

            # ================= SETUP DMA =================
            # priority order: step-1 inputs first, then the recurrence weights
            # (needed ~25us in, during AG#1), then proj weights.
            sync.dma_start(zT_sb[:], zT_in.ap().rearrange("(k p) b -> p k b", p=128))
            sync.dma_start(w0T_sb[:], w0T_in.ap().rearrange("(k p) c -> p k c", p=128))
            sync.dma_start(vecs[:, BH:BH + 6], bh_in.ap().rearrange("(k p) -> p k", p=128))
            sync.dma_start(vecs[:, BRZ:BRZ + 4], brz_in.ap().rearrange("(k p) -> p k", p=128))
            sync.dma_start(vecs[:, CIN:CIN + 2], cin_in.ap().rearrange("(k p) -> p k", p=128))
            sync.dma_start(vecs[:, BRZ1:BRZ1 + 4], brz1_in.ap().rearrange("(k p) -> p k", p=128))
            sync.dma_start(vecs[:, C1N:C1N + 2], c1n_in.ap().rearrange("(k p) -> p k", p=128))
            sync.dma_start(lb_sb[:], linbs_in.ap())
            nc.gpsimd.memset(magic_sb[:], 0x5f3759df)
            make_identity(nc, ident[:])
            if USE_F8:
                for j in range(2):  # 16*b_hh n-part for the fused hn scale trick
                    nc.vector.tensor_scalar(vecs[:, BH16 + j:BH16 + j + 1],
                                            vecs[:, BH + 4 + j:BH + 5 + j],
                                            SW, None, OP.mult)
            nc.scalar.dma_start(W_all[:], wcT_in.ap().rearrange("(k p) c -> p k c", p=128))
            nc.scalar.dma_start(linwT_sb[:], linwT_in.ap().rearrange("(k p) c -> p k c", p=128))

            # ================= STEP 1 (from z) =================
            with (
                tc.tile_pool(name="spn", bufs=2, space="PSUM") as spn,
            ):
                # q = W0_s @ z.T ; gates with h0 = 0
                qps = []
                for m in range(6):
                    qp = spn.tile([128, B], f32, tag="pp")
                    for k in range(4):
                        nc.tensor.matmul(qp[:], w0T_sb[:, k, ts(m, 128)], zT_sb[:, k, :],
                                         start=(k == 0), stop=(k == 3))
                    qps.append(qp)
                    if m % 2 == 1:
                        for j in (m - 1, m):
                            g = j % 2  # unit chunk
                            if m == 1:  # r gates
                                nc.scalar.activation(r_sb[:, g, :], qps[j][:], AF.Sigmoid,
                                                     bias=vecs[:, BRZ1 + j:BRZ1 + j + 1],
                                                     scale=ISW)
                            elif m == 3:  # z gates
                                nc.scalar.activation(u_sb[:, g, :], qps[j][:], AF.Sigmoid,
                                                     bias=vecs[:, BRZ1 + j:BRZ1 + j + 1],
                                                     scale=ISW)
                            else:  # n gates: i_n = q/SW + c1_n
                                nc.vector.tensor_scalar(in_sb[:, g, :], qps[j][:],
                                                        ISW,
                                                        vecs[:, C1N + g:C1N + g + 1],
                                                        OP.mult, OP.add)
                for j in range(2):
                    # n = tanh(i_n + r * b_hh_n);  h1 = n - u*n
                    nc.vector.scalar_tensor_tensor(pre_sb[:, j, :], r_sb[:, j, :],
                                                   vecs[:, BH + 4 + j:BH + 5 + j],
                                                   in_sb[:, j, :], OP.mult, OP.add)
                    nc.scalar.activation(d_sb[:, j, :], pre_sb[:, j, :], AF.Tanh)
                    nc.vector.tensor_tensor(e_sb[:, j, :], u_sb[:, j, :], d_sb[:, j, :], OP.mult)
                    nc.vector.tensor_tensor(h_new[:, j, :], d_sb[:, j, :], e_sb[:, j, :], OP.subtract)
                    nc.vector.tensor_copy(h_new_x[:, j, :], h_new[:, j, :])
                    sync.dma_start(cc_in[0].ap().rearrange("(j p) b -> p j b", p=128)[:, j:j+1, :],
                                   h_new_x[:, j:j+1, :])
                nc.gpsimd.collective_compute("AllGather", OP.bypass, replica_groups=rg,
                                             ins=[cc_in[0].ap().opt()], outs=[cc_out[0].ap().opt()])
                sync.dma_start(h_T[:], cc_out[0].ap().rearrange("(k p) b -> p k b", p=128))

            # ================= RECURRENCE + OUTPUT =================
            with (
                tc.tile_pool(name="loop_sb", bufs=1) as lsb,
                tc.tile_pool(name="gp", bufs=6, space="PSUM") as gp,
                tc.tile_pool(name="op", bufs=1, space="PSUM") as opp,
                tc.tile_pool(name="trp", bufs=1, space="PSUM") as trp,
            ):

                TLIM = int(os.environ.get("BASS_T_LIM", str(T)))

                def proj_block(s):
                    """x_s = lin_w_slice @ h_s (+lin_b), BatchNorm, transpose, store.
                    Reads h_s from h_T during the AllGather window; the return
                    DMA overwrites h_T only after these MMs retire."""
                    xp = opp.tile([ZS, B], f32, tag="xp")
                    for k in range(KC):
                        nc.tensor.matmul(xp[:], linwT_sb[:, k, :], h_T[:, k, :],
                                         start=(k == 0), stop=(k == KC - 1))
                    x_sb = lsb.tile([ZS, B], f32, tag="x_sb", name="x_sb", bufs=2)
                    st = lsb.tile([ZS, 8], f32, tag="st", name="st", bufs=2)
                    sc_sb = lsb.tile([ZS, B], f32, tag="sc_sb", name="sc_sb", bufs=2)
                    y_sb = lsb.tile([ZS, B], f32, tag="y_sb", name="y_sb", bufs=2)
                    ybm = lsb.tile([128, 2, ZS], f32, tag="ybm", name="ybm", bufs=2)
                    rs = lsb.tile([ZS, 8], f32, tag="rs", name="rs", bufs=2)
                    nc.vector.tensor_scalar(x_sb[:], xp[:], ISW, lb_sb[:],
                                            OP.mult, OP.add)
                    nc.vector.tensor_reduce(st[:, 0:1], x_sb[:],
                                            mybir.AxisListType.X, OP.add)
                    nc.vector.tensor_tensor(sc_sb[:], x_sb[:], x_sb[:], OP.mult)
                    nc.vector.tensor_reduce(st[:, 1:2], sc_sb[:],
                                            mybir.AxisListType.X, OP.add)
                    nc.vector.tensor_scalar(st[:, 2:3], st[:, 0:1], 1.0 / B, None, OP.mult)
                    nc.vector.tensor_scalar(st[:, 3:4], st[:, 1:2], 1.0 / B, None, OP.mult)
                    nc.vector.scalar_tensor_tensor(st[:, 4:5], st[:, 2:3], st[:, 2:3],
                                                   st[:, 3:4], OP.mult, OP.subtract)
                    nc.vector.tensor_scalar(st[:, 5:6], st[:, 4:5], -1.0, EPS,
                                            OP.mult, OP.add)       # var + eps
                    _emit_rsqrt(nc, st[:, 6:7], st[:, 5:6], magic_sb, rs[:])
                    nc.vector.tensor_scalar(y_sb[:], x_sb[:], st[:, 2:3], st[:, 6:7],
                                            OP.subtract, OP.mult)
                    for bc in range(2):
                        tp = trp.tile([128, ZS], f32, tag="tp")
                        nc.tensor.transpose(tp[:], y_sb[:, ts(bc, 128)], ident[0:ZS, 0:ZS])
                        nc.vector.tensor_copy(ybm[:, bc, :], tp[:])
                    nc.scalar.dma_start(y_out.ap()[s - 1, :, :]
                                        .rearrange("(bc p) z -> p bc z", p=128), ybm[:])

                def gates(s, j):
                    gps = gtiles[j]  # r, hn, in, z
                    bh_col = BH16 + j if USE_F8 else BH + 4 + j
                    nc.scalar.activation(r_sb[:, j, :], gps[0][:], AF.Sigmoid,
                                         bias=vecs[:, BRZ + j:BRZ + j + 1],
                                         scale=ISW)
                    nc.vector.scalar_tensor_tensor(pre_sb[:, j, :], gps[1][:],
                                                   vecs[:, bh_col:bh_col + 1],
                                                   r_sb[:, j, :], OP.add, OP.mult)
                    nc.vector.tensor_tensor(in_sb[:, j, :], pre_sb[:, j, :],
                                            gps[2][:], OP.add)
                    nc.scalar.activation(d_sb[:, j, :], in_sb[:, j, :], AF.Tanh,
                                         bias=vecs[:, CIN + j:CIN + j + 1],
                                         scale=ISW)
                    nc.scalar.activation(u_sb[:, j, :], gps[3][:], AF.Sigmoid,
                                         bias=vecs[:, BRZ + 2 + j:BRZ + 3 + j],
                                         scale=ISW)
                    # h_new = n + u*(h_prev - n); exchange copy written first
                    nc.vector.tensor_tensor(e_sb[:, j, :], h_new[:, j, :],
                                            d_sb[:, j, :], OP.subtract)
                    nc.vector.tensor_tensor(pre_sb[:, j, :], u_sb[:, j, :],
                                            e_sb[:, j, :], OP.mult)
                    nc.vector.tensor_tensor(h_new_x[:, j, :], d_sb[:, j, :],
                                            pre_sb[:, j, :], OP.add)
                    sync.dma_start(cc_in[s - 1].ap()
                                   .rearrange("(j p) b -> p j b", p=128)[:, j:j+1, :],
                                   h_new_x[:, j:j+1, :])
                    nc.vector.tensor_tensor(h_new[:, j, :], d_sb[:, j, :],
                                            pre_sb[:, j, :], OP.add)

                for s in range(2, TLIM + 1):  # steps 2..TLIM, h_{s-1} -> h_s
                    gtiles = [[], []]
                    for j in range(2):
                        # m-chunk roles for unit chunk j: r, hn, in, z
                        for m in (j, 6 + j, 4 + j, 2 + j):
                            g = gp.tile([128, B], f32, tag="g")
                            gtiles[j].append(g)
                            for k in range(KC):
                                nc.tensor.matmul(g[:], W_all[:, k, ts(m, 128)], h_T[:, k, :],
                                                 start=(k == 0), stop=(k == KC - 1))
                        gates(s, j)
                    nc.gpsimd.collective_compute("AllGather", OP.bypass, replica_groups=rg,
                                                 ins=[cc_in[s - 1].ap().opt()],
                                                 outs=[cc_out[s - 1].ap().opt()])
                    proj_block(s - 1)  # rides the gather window, reads h_T pre-return
                    cco_r = cc_out[s - 1].ap().rearrange("(k p) b -> p k b", p=128)
                    sync.dma_start(h_T[:], cco_r)
                proj_block(TLIM)

    nc.compile()
    return nc


_NC_CACHE = [None]


def kernel(z, fc_w, fc_b, fc_u, w_ih, w_hh, b_ih, b_hh, lin_w, lin_b):
    z = np.asarray(z, dtype=np.float32)
    fc_w = np.asarray(fc_w, dtype=np.float32)
    fc_b = np.asarray(fc_b, dtype=np.float32)
    fc_u = np.asarray(fc_u, dtype=np.float32)
    w_ih = np.asarray(w_ih, dtype=np.float32)
    w_hh = np.asarray(w_hh, dtype=np.float32)
    b_ih = np.asarray(b_ih, dtype=np.float32)
    b_hh = np.asarray(b_hh, dtype=np.float32)
    lin_w = np.asarray(lin_w, dtype=np.float32)
    lin_b = np.asarray(lin_b, dtype=np.float32)

    if USE_F8:
        wdt = mybir.dt.np(mybir.dt.float8e4)
    else:
        wdt = ml_dtypes.bfloat16 if USE_BF16 else np.float16

    # ---- host-side weight algebra (z-independent) ----
    # spectral norm sigma (one torch-style power iteration, u/v constants)
    v = fc_w.T @ fc_u
    v = v / (np.linalg.norm(v) + 1e-12)
    wv = fc_w @ v
    u1 = wv / (np.linalg.norm(wv) + 1e-12)
    sigma = float(u1 @ wv)

    # step-1 fused input weight: gi_0 = z @ W0.T + b0
    W0 = (w_ih @ fc_w) / sigma                  # [3H, Z]
    b0 = b_ih + w_ih @ fc_b                     # [3H]
    # steps>=2 fused weights: gi_t = h @ (w_ih @ lin_w).T + ci
    Wf = w_ih @ lin_w                           # [3H, H]
    ci = b_ih + w_ih @ lin_b                    # [3H]
    W_rz = Wf[:2 * H] + w_hh[:2 * H]            # fused r/z (sigmoid args add)
    W_in = Wf[2 * H:]
    W_hn = w_hh[2 * H:]

    W04 = W0.reshape(3, NC, HS, Z)
    b04 = b0.reshape(3, NC, HS)
    ci3 = ci.reshape(3, NC, HS)
    bh3 = b_hh.reshape(3, NC, HS)
    Wrz4 = W_rz.reshape(2, NC, HS, H)
    Win4 = W_in.reshape(NC, HS, H)
    Whn4 = W_hn.reshape(NC, HS, H)
    lin_wT = lin_w.T                            # [H, Z]

    zT = np.ascontiguousarray(z.T).astype(wdt)

    in_maps = []
    for c in range(NC):
        w0s = W04[:, c].reshape(GR, Z) * SW     # r,z,n rows for this core
        # W_comb column layout: [r, z] fused | in | hn   (FR = 4*HS)
        wc = np.concatenate([Wrz4[0, c], Wrz4[1, c], Win4[c], Whn4[c]], axis=0) * SW  # [FR, H]
        brz = np.concatenate([ci3[0, c] + bh3[0, c], ci3[1, c] + bh3[1, c]])     # [2*HS]
        cin = ci3[2, c]                                                          # [HS]
        brz1 = np.concatenate([b04[0, c] + bh3[0, c], b04[1, c] + bh3[1, c]])
        c1n = b04[2, c]
        bh = bh3[:, c].reshape(GR)
        in_maps.append({
            "zT": zT,
            "w0T_s": np.ascontiguousarray(w0s.T).astype(wdt),
            "wcT_s": np.ascontiguousarray(wc.T).astype(wdt),
            "lin_wT_s": np.ascontiguousarray(lin_wT[:, c * ZS:(c + 1) * ZS] * SW).astype(wdt),
            "lin_b_s": np.ascontiguousarray(lin_b[c * ZS:(c + 1) * ZS].reshape(ZS, 1)),
            "bh_s": np.ascontiguousarray(bh),
            "brz_s": np.ascontiguousarray(brz),
            "cin_s": np.ascontiguousarray(cin),
            "brz1_s": np.ascontiguousarray(brz1),
            "c1n_s": np.ascontiguousarray(c1n),
        })

    if _NC_CACHE[0] is None:
        _NC_CACHE[0] = build_nc()
    nc = _NC_CACHE[0]

    trace = os.environ.get("BASS_KERNEL_TRACE") == "1"
    if trace:
        _install_ntff_hook()
    res = run_bass_kernel_spmd(nc, in_maps, core_ids=list(range(NC)), trace=trace)
    LAST_EXEC_NS[0] = res.exec_time_ns
    LAST_RESULTS[0] = res

    full = np.empty((T, B, Z), dtype=np.float32)
    for c in range(NC):
        full[:, :, c * ZS:(c + 1) * ZS] = res.results[c]["y_part"]
    return full.transpose(1, 0, 2).reshape(B * T, Z)


# revision 17
# speedup vs baseline: 1.0587x; 1.0587x over previous
"""Trainium2 Bass kernel for nn_CGRU (spectral-norm linear -> GRU x16 -> per-step
BatchNorm), 8-way model-parallel over the hidden dimension.

Shapes (hardcoded): B=256, Z=512, H=2048, T=16, 8 cores.

v2: all weight algebra (spectral-norm sigma, lin/fc folds into the GRU input
weights, bias folds) is done on the host once per call; the device runs only
the z-dependent recurrence:
  step 1:   gates = W0 @ z.T            (W0 = w_ih @ fc_w / sigma, host-folded)
  steps 2+: gates = W_comb @ h.T        (W_comb = w_ih @ lin_w (+ w_hh), folded)
  per step: 8-core AllGather of the h shard (f16), proj/BN of the previous h
            ride the gather window.
"""
import os
import sys
import types
import contextlib
import ctypes

import numpy as np
import ml_dtypes

import concourse.bass as bass
import concourse.bacc as bacc
import concourse.mybir as mybir
import concourse.tile as tile
from concourse.bass import ts
from concourse.bass_utils import run_bass_kernel_spmd
from concourse.masks import make_identity

f32 = mybir.dt.float32
f32r = mybir.dt.float32r
bf16 = mybir.dt.bfloat16
fp16 = mybir.dt.float16
AF = mybir.ActivationFunctionType
OP = mybir.AluOpType

B, Z, H, T, NC = 256, 512, 2048, 16, 8
HS = H // NC          # 256 hidden units per core (2 chunks of 128)
GR = 3 * HS           # 768 gate rows per core (r,z,n)
FR = 4 * HS           # 1024 fused rows per core (rz fused, in, hn)
ZS = Z // NC          # 64 output features per core
KC = H // 128         # 16 contraction chunks
EPS = 1e-5

XDT = os.environ.get("BASS_XDT", "f16")   # f16 | bf16 | f8 (f8 fails tolerance)
USE_BF16 = XDT == "bf16"
USE_F8 = XDT == "f8"
SW = 16.0 if USE_F8 else 1.0              # host-side weight scale (fp8 range)
ISW = 1.0 / SW

# vecs column map ([128, 64] fp32 scratch of per-partition scalars)
# bh_s[768] -> BH..BH+5 ; brz_s[512] -> BRZ.. ; cin_s[256] -> CIN.. ;
# brz1_s[512] -> BRZ1.. ; c1n_s[256] -> C1N.. ; BH16: 16*b_hh n-part (f8)
BH, BRZ, CIN, BRZ1, C1N, BH16 = 0, 6, 10, 12, 16, 18

LAST_EXEC_NS = [None]
LAST_RESULTS = [None]


def _install_ntff_hook():
    """The agent image lacks antenv.axon_hooks; recreate it so
    run_bass_kernel_spmd(trace=True) can capture NTFF profiles via the
    libaxon_pjrt.so C ABI (same as trn_agent_boot)."""
    try:
        import antenv
    except ImportError:
        return
    if "antenv.axon_hooks" in sys.modules:
        return
    so_path = "/opt/axon/libaxon_pjrt.so"
    if not os.path.exists(so_path):
        return
    lib = ctypes.CDLL(so_path)
    if not hasattr(lib, "axon_start_nrt_profile"):
        return
    lib.axon_start_nrt_profile.argtypes = [ctypes.POINTER(ctypes.c_int64), ctypes.c_size_t]
    lib.axon_start_nrt_profile.restype = ctypes.c_int64
    lib.axon_stop_nrt_profile.argtypes = [ctypes.c_char_p]
    lib.axon_stop_nrt_profile.restype = ctypes.c_int64

    @contextlib.contextmanager
    def _hook(output_dir, device_ids):
        import jax

        jax.devices()
        if device_ids:
            ids = (ctypes.c_int64 * len(device_ids))(*device_ids)
            rc = lib.axon_start_nrt_profile(ids, len(device_ids))
        else:
            rc = lib.axon_start_nrt_profile(None, 0)
        if rc != 0:
            raise RuntimeError(f"axon_start_nrt_profile rc={rc}")
        try:
            yield
        finally:
            n = lib.axon_stop_nrt_profile(str(output_dir).encode())
            print(f"profile: {n} file(s) written to {output_dir}", file=sys.stderr)

    mod = types.ModuleType("antenv.axon_hooks")
    _state = {"hook": _hook}
    mod.set_axon_ntff_profile_hook = lambda h: _state.__setitem__("hook", h)
    mod.get_axon_ntff_profile_hook = lambda: _state["hook"]
    sys.modules["antenv.axon_hooks"] = mod
    antenv.axon_hooks = mod


def _emit_rsqrt(nc, out_ap, v_ap, magic_ap, scr):
    """out = 1/sqrt(v) via bit-trick seed + 3 Newton iterations (DVE only).
    scr: [P, 8] fp32 scratch tile AP (cols 0..5 used)."""
    i32 = mybir.dt.int32
    P = v_ap.shape[0]
    c = lambda k: scr[0:P, k:k + 1]
    nc.vector.tensor_scalar(c(0).bitcast(i32), v_ap.bitcast(i32), 1, None,
                            OP.arith_shift_right)
    nc.vector.tensor_tensor(c(1).bitcast(i32), magic_ap[0:P, :], c(0).bitcast(i32),
                            OP.subtract)                      # y0
    nc.vector.tensor_scalar(c(2), v_ap, 0.5, None, OP.mult)   # hv
    ycols = (1, 5, 1)
    for it in range(3):
        y = c(ycols[it])
        nc.vector.scalar_tensor_tensor(c(3), y, c(2), y, OP.mult, OP.mult)  # p = y*hv*y
        nc.vector.tensor_scalar(c(4), c(3), -1.0, 1.5, OP.mult, OP.add)     # q = 1.5 - p
        dst = out_ap if it == 2 else c(ycols[it + 1])
        nc.vector.tensor_tensor(dst, y, c(4), OP.mult)


def build_nc():
    DT = mybir.dt.float8e4 if USE_F8 else (bf16 if USE_BF16 else fp16)

    nc = bacc.Bacc("TRN2", target_bir_lowering=False, debug=False, num_devices=NC)

    # ---- I/O (all weights host-folded) ----
    zT_in = nc.dram_tensor("zT", [Z, B], DT, kind="ExternalInput")
    w0T_in = nc.dram_tensor("w0T_s", [Z, GR], DT, kind="ExternalInput")
    wcT_in = nc.dram_tensor("wcT_s", [H, FR], DT, kind="ExternalInput")
    linwT_in = nc.dram_tensor("lin_wT_s", [H, ZS], DT, kind="ExternalInput")
    linbs_in = nc.dram_tensor("lin_b_s", [ZS, 1], f32, kind="ExternalInput")
    bh_in = nc.dram_tensor("bh_s", [GR], f32, kind="ExternalInput")
    brz_in = nc.dram_tensor("brz_s", [4 * 128], f32, kind="ExternalInput")
    cin_in = nc.dram_tensor("cin_s", [2 * 128], f32, kind="ExternalInput")
    brz1_in = nc.dram_tensor("brz1_s", [4 * 128], f32, kind="ExternalInput")
    c1n_in = nc.dram_tensor("c1n_s", [2 * 128], f32, kind="ExternalInput")
    y_out = nc.dram_tensor("y_part", [T, B, ZS], f32, kind="ExternalOutput")

    # per-step collective bounce buffers (ring of NB, reused across steps)
    NB = int(os.environ.get("BASS_CC_BUFS", "4"))
    cc_in = [nc.dram_tensor(f"cc_in{t}", [HS, B], DT) for t in range(NB)]
    cc_out = [
        nc.dram_tensor(f"cc_out{t}", [H, B], DT, addr_space="Shared")
        for t in range(NB)
    ]
    cc_in = [cc_in[t % NB] for t in range(T)]
    cc_out = [cc_out[t % NB] for t in range(T)]
    ccw_in = nc.dram_tensor("ccw_in", [8, 16], f32)
    ccw_out = nc.dram_tensor("ccw_out", [64, 16], f32, addr_space="Shared")
    rg = [list(range(NC))]

    with tile.TileContext(nc) as tc:
        with tc.tile_pool(name="perm", bufs=1) as perm:
            # fire a tiny AllGather immediately: ncfw first-collective warm-up
            # then runs concurrently with the setup DMA/compute.
            nc.gpsimd.collective_compute("AllGather", OP.bypass, replica_groups=rg,
                                         ins=[ccw_in.ap().opt()],
                                         outs=[ccw_out.ap().opt()])
            # ---- persistent SBUF ----
            W_all = perm.tile([128, KC, FR], DT, name="W_all")
            w0T_sb = perm.tile([128, 4, GR], DT, name="w0T_sb")
            linwT_sb = perm.tile([128, KC, ZS], DT, name="linwT_sb")
            h_T = perm.tile([128, KC, B], DT, name="h_T")
            h_new = perm.tile([128, 2, B], f32, name="h_new")
            h_new_x = perm.tile([128, 2, B], DT, name="h_new_x")
            vecs = perm.tile([128, 24], f32, name="vecs")
            ident = perm.tile([128, 128], f32, name="ident")
            lb_sb = perm.tile([ZS, 1], f32, name="lb_sb")
            magic_sb = perm.tile([128, 1], mybir.dt.int32, name="magic_sb")
            zT_sb = perm.tile([128, 4, B], DT, name="zT_sb")
            # gate work tiles (single-buffered, reused every step)
            r_sb = perm.tile([128, 2, B], f32, name="r_sb")
            u_sb = perm.tile([128, 2, B], f32, name="u_sb")
            in_sb = perm.tile([128, 2, B], f32, name="in_sb")
            pre_sb = perm.tile([128, 2, B], f32, name="pre_sb")
            d_sb = perm.tile([128, 2, B], f32, name="d_sb")
            e_sb = perm.tile([128, 2, B], f32, name="e_sb")

            sync = nc.sync

            # ================= SETUP DMA =================
            # priority order: step-1 inputs first, then the recurrence weights
            # (needed ~25us in, during AG#1), then proj weights.
            sync.dma_start(zT_sb[:], zT_in.ap().rearrange("(k p) b -> p k b", p=128))
            sync.dma_start(w0T_sb[:], w0T_in.ap().rearrange("(k p) c -> p k c", p=128))
            sync.dma_start(vecs[:, BH:BH + 6], bh_in.ap().rearrange("(k p) -> p k", p=128))
            sync.dma_start(vecs[:, BRZ:BRZ + 4], brz_in.ap().rearrange("(k p) -> p k", p=128))
            sync.dma_start(vecs[:, CIN:CIN + 2], cin_in.ap().rearrange("(k p) -> p k", p=128))
            sync.dma_start(vecs[:, BRZ1:BRZ1 + 4], brz1_in.ap().rearrange("(k p) -> p k", p=128))
            sync.dma_start(vecs[:, C1N:C1N + 2], c1n_in.ap().rearrange("(k p) -> p k", p=128))
            sync.dma_start(lb_sb[:], linbs_in.ap())
            nc.gpsimd.memset(magic_sb[:], 0x5f3759df)
            make_identity(nc, ident[:])
            if USE_F8:
                for j in range(2):  # 16*b_hh n-part for the fused hn scale trick
                    nc.vector.tensor_scalar(vecs[:, BH16 + j:BH16 + j + 1],
                                            vecs[:, BH + 4 + j:BH + 5 + j],
                                            SW, None, OP.mult)
            nc.scalar.dma_start(W_all[:], wcT_in.ap().rearrange("(k p) c -> p k c", p=128))
            nc.scalar.dma_start(linwT_sb[:], linwT_in.ap().rearrange("(k p) c -> p k c", p=128))

            # ================= STEP 1 (from z) =================
            with (
                tc.tile_pool(name="spn", bufs=2, space="PSUM") as spn,
            ):
                # q = W0_s @ z.T ; gates with h0 = 0
                qps = []
                for m in range(6):
                    qp = spn.tile([128, B], f32, tag="pp")
                    for k in range(4):
                        nc.tensor.matmul(qp[:], w0T_sb[:, k, ts(m, 128)], zT_sb[:, k, :],
                                         start=(k == 0), stop=(k == 3))
                    qps.append(qp)
                    if m % 2 == 1:
                        for j in (m - 1, m):
                            g = j % 2  # unit chunk
                            if m == 1:  # r gates
                                nc.scalar.activation(r_sb[:, g, :], qps[j][:], AF.Sigmoid,
                                                     bias=vecs[:, BRZ1 + j:BRZ1 + j + 1],
                                                     scale=ISW)
                            elif m == 3:  # z gates
                                nc.scalar.activation(u_sb[:, g, :], qps[j][:], AF.Sigmoid,
                                                     bias=vecs[:, BRZ1 + j:BRZ1 + j + 1],
                                                     scale=ISW)
                            else:  # n gates: i_n = q/SW + c1_n
                                nc.vector.tensor_scalar(in_sb[:, g, :], qps[j][:],
                                                        ISW,
                                                        vecs[:, C1N + g:C1N + g + 1],
                                                        OP.mult, OP.add)
                for j in range(2):
                    # n = tanh(i_n + r * b_hh_n);  h1 = n - u*n
                    nc.vector.scalar_tensor_tensor(pre_sb[:, j, :], r_sb[:, j, :],
                                                   vecs[:, BH + 4 + j:BH + 5 + j],
                                                   in_sb[:, j, :], OP.mult, OP.add)
                    nc.scalar.activation(d_sb[:, j, :], pre_sb[:, j, :], AF.Tanh)
                    nc.vector.tensor_tensor(e_sb[:, j, :], u_sb[:, j, :], d_sb[:, j, :], OP.mult)
                    nc.vector.tensor_tensor(h_new[:, j, :], d_sb[:, j, :], e_sb[:, j, :], OP.subtract)
                    nc.vector.tensor_copy(h_new_x[:, j, :], h_new[:, j, :])
                    sync.dma_start(cc_in[0].ap().rearrange("(j p) b -> p j b", p=128)[:, j:j+1, :],
                                   h_new_x[:, j:j+1, :])
                nc.gpsimd.collective_compute("AllGather", OP.bypass, replica_groups=rg,
                                             ins=[cc_in[0].ap().opt()], outs=[cc_out[0].ap().opt()])
                sync.dma_start(h_T[:], cc_out[0].ap().rearrange("(k p) b -> p k b", p=128))

            # ================= RECURRENCE + OUTPUT =================
            with (
                tc.tile_pool(name="loop_sb", bufs=1) as lsb,
                tc.tile_pool(name="gp", bufs=6, space="PSUM") as gp,
                tc.tile_pool(name="op", bufs=1, space="PSUM") as opp,
                tc.tile_pool(name="trp", bufs=1, space="PSUM") as trp,
            ):

                TLIM = int(os.environ.get("BASS_T_LIM", str(T)))

                def proj_block(s):
                    """x_s = lin_w_slice @ h_s (+lin_b), BatchNorm, transpose, store.
                    Reads h_s from h_T during the AllGather window; the return
                    DMA overwrites h_T only after these MMs retire."""
                    xp = opp.tile([ZS, B], f32, tag="xp")
                    for k in range(KC):
                        nc.tensor.matmul(xp[:], linwT_sb[:, k, :], h_T[:, k, :],
                                         start=(k == 0), stop=(k == KC - 1))
                    x_sb = lsb.tile([ZS, B], f32, tag="x_sb", name="x_sb", bufs=2)
                    st = lsb.tile([ZS, 8], f32, tag="st", name="st", bufs=2)
                    sc_sb = lsb.tile([ZS, B], f32, tag="sc_sb", name="sc_sb", bufs=2)
                    y_sb = lsb.tile([ZS, B], f32, tag="y_sb", name="y_sb", bufs=2)
                    ybm = lsb.tile([128, 2, ZS], f32, tag="ybm", name="ybm", bufs=2)
                    rs = lsb.tile([ZS, 8], f32, tag="rs", name="rs", bufs=2)
                    nc.vector.tensor_scalar(x_sb[:], xp[:], ISW, lb_sb[:],
                                            OP.mult, OP.add)
                    nc.vector.tensor_reduce(st[:, 0:1], x_sb[:],
                                            mybir.AxisListType.X, OP.add)
                    nc.vector.tensor_tensor(sc_sb[:], x_sb[:], x_sb[:], OP.mult)
                    nc.vector.tensor_reduce(st[:, 1:2], sc_sb[:],
                                            mybir.AxisListType.X, OP.add)
                    nc.vector.tensor_scalar(st[:, 2:3], st[:, 0:1], 1.0 / B, None, OP.mult)
                    nc.vector.tensor_scalar(st[:, 3:4], st[:, 1:2], 1.0 / B, None, OP.mult)
                    nc.vector.scalar_tensor_tensor(st[:, 4:5], st[:, 2:3], st[:, 2:3],
                                                   st[:, 3:4], OP.mult, OP.subtract)
                    nc.vector.tensor_scalar(st[:, 5:6], st[:, 4:5], -1.0, EPS,
                                            OP.mult, OP.add)       # var + eps
                    _emit_rsqrt(nc, st[:, 6:7], st[:, 5:6], magic_sb, rs[:])
                    nc.vector.tensor_scalar(y_sb[:], x_sb[:], st[:, 2:3], st[:, 6:7],
                                            OP.subtract, OP.mult)
                    for bc in range(2):
                        tp = trp.tile([128, ZS], f32, tag="tp")
                        nc.tensor.transpose(tp[:], y_sb[:, ts(bc, 128)], ident[0:ZS, 0:ZS])
                        nc.vector.tensor_copy(ybm[:, bc, :], tp[:])
                    nc.scalar.dma_start(y_out.ap()[s - 1, :, :]
                                        .rearrange("(bc p) z -> p bc z", p=128), ybm[:])

                def gates(s, j):
                    gps = gtiles[j]  # APs: r, hn, in, z
                    bh_col = BH16 + j if USE_F8 else BH + 4 + j
                    nc.scalar.activation(r_sb[:, j, :], gps[0], AF.Sigmoid,
                                         bias=vecs[:, BRZ + j:BRZ + j + 1],
                                         scale=ISW)
                    nc.vector.scalar_tensor_tensor(pre_sb[:, j, :], gps[1],
                                                   vecs[:, bh_col:bh_col + 1],
                                                   r_sb[:, j, :], OP.add, OP.mult)
                    nc.vector.tensor_tensor(in_sb[:, j, :], pre_sb[:, j, :],
                                            gps[2], OP.add)
                    nc.scalar.activation(d_sb[:, j, :], in_sb[:, j, :], AF.Tanh,
                                         bias=vecs[:, CIN + j:CIN + j + 1],
                                         scale=ISW)
                    nc.scalar.activation(u_sb[:, j, :], gps[3], AF.Sigmoid,
                                         bias=vecs[:, BRZ + 2 + j:BRZ + 3 + j],
                                         scale=ISW)
                    # h_new = n + u*(h_prev - n); exchange copy written first
                    nc.vector.tensor_tensor(e_sb[:, j, :], h_new[:, j, :],
                                            d_sb[:, j, :], OP.subtract)
                    nc.vector.tensor_tensor(pre_sb[:, j, :], u_sb[:, j, :],
                                            e_sb[:, j, :], OP.mult)
                    nc.vector.tensor_tensor(h_new_x[:, j, :], d_sb[:, j, :],
                                            pre_sb[:, j, :], OP.add)
                    sync.dma_start(cc_in[s - 1].ap()
                                   .rearrange("(j p) b -> p j b", p=128)[:, j:j+1, :],
                                   h_new_x[:, j:j+1, :])
                    nc.vector.tensor_tensor(h_new[:, j, :], d_sb[:, j, :],
                                            pre_sb[:, j, :], OP.add)

                # m-chunk emission order: j=0's four roles first so its gate
                # elementwise overlaps j=1's matmuls
                MORD = (0, 6, 4, 2, 1, 7, 5, 3)

                for s in range(2, TLIM + 1):  # steps 2..TLIM, h_{s-1} -> h_s
                    gtiles = [[], []]
                    for j in range(2):
                        # m-chunk roles for unit chunk j: r, hn, in, z
                        for m in (j, 6 + j, 4 + j, 2 + j):
                            g_t = gp.tile([128, B], f32, tag="g", name="g_t")
                            gtiles[j].append(g_t[:])
                            for k in range(KC):
                                nc.tensor.matmul(g_t[:], W_all[:, k, ts(m, 128)],
                                                 h_T[:, k, :],
                                                 start=(k == 0), stop=(k == KC - 1))
                        gates(s, j)
                    nc.gpsimd.collective_compute("AllGather", OP.bypass, replica_groups=rg,
                                                 ins=[cc_in[s - 1].ap().opt()],
                                                 outs=[cc_out[s - 1].ap().opt()])
                    proj_block(s - 1)  # rides the gather window, reads h_T pre-return
                    cco_r = cc_out[s - 1].ap().rearrange("(k p) b -> p k b", p=128)
                    sync.dma_start(h_T[:, 0:KC // 2, :], cco_r[:, 0:KC // 2, :])
                    nc.scalar.dma_start(h_T[:, KC // 2:, :], cco_r[:, KC // 2:, :])
                proj_block(TLIM)

    nc.compile()
    return nc


_NC_CACHE = [None]


def kernel(z, fc_w, fc_b, fc_u, w_ih, w_hh, b_ih, b_hh, lin_w, lin_b):
    z = np.asarray(z, dtype=np.float32)
    fc_w = np.asarray(fc_w, dtype=np.float32)
    fc_b = np.asarray(fc_b, dtype=np.float32)
    fc_u = np.asarray(fc_u, dtype=np.float32)
    w_ih = np.asarray(w_ih, dtype=np.float32)
    w_hh = np.asarray(w_hh, dtype=np.float32)
    b_ih = np.asarray(b_ih, dtype=np.float32)
    b_hh = np.asarray(b_hh, dtype=np.float32)
    lin_w = np.asarray(lin_w, dtype=np.float32)
    lin_b = np.asarray(lin_b, dtype=np.float32)

    if USE_F8:
        wdt = mybir.dt.np(mybir.dt.float8e4)
    else:
        wdt = ml_dtypes.bfloat16 if USE_BF16 else np.float16

    # ---- host-side weight algebra (z-independent) ----
    # spectral norm sigma (one torch-style power iteration, u/v constants)
    v = fc_w.T @ fc_u
    v = v / (np.linalg.norm(v) + 1e-12)
    wv = fc_w @ v
    u1 = wv / (np.linalg.norm(wv) + 1e-12)
    sigma = float(u1 @ wv)

    # step-1 fused input weight: gi_0 = z @ W0.T + b0
    W0 = (w_ih @ fc_w) / sigma                  # [3H, Z]
    b0 = b_ih + w_ih @ fc_b                     # [3H]
    # steps>=2 fused weights: gi_t = h @ (w_ih @ lin_w).T + ci
    Wf = w_ih @ lin_w                           # [3H, H]
    ci = b_ih + w_ih @ lin_b                    # [3H]
    W_rz = Wf[:2 * H] + w_hh[:2 * H]            # fused r/z (sigmoid args add)
    W_in = Wf[2 * H:]
    W_hn = w_hh[2 * H:]

    W04 = W0.reshape(3, NC, HS, Z)
    b04 = b0.reshape(3, NC, HS)
    ci3 = ci.reshape(3, NC, HS)
    bh3 = b_hh.reshape(3, NC, HS)
    Wrz4 = W_rz.reshape(2, NC, HS, H)
    Win4 = W_in.reshape(NC, HS, H)
    Whn4 = W_hn.reshape(NC, HS, H)
    lin_wT = lin_w.T                            # [H, Z]

    zT = np.ascontiguousarray(z.T).astype(wdt)

    in_maps = []
    for c in range(NC):
        w0s = W04[:, c].reshape(GR, Z) * SW     # r,z,n rows for this core
        # W_comb column layout: [r, z] fused | in | hn   (FR = 4*HS)
        wc = np.concatenate([Wrz4[0, c], Wrz4[1, c], Win4[c], Whn4[c]], axis=0) * SW  # [FR, H]
        brz = np.concatenate([ci3[0, c] + bh3[0, c], ci3[1, c] + bh3[1, c]])     # [2*HS]
        cin = ci3[2, c]                                                          # [HS]
        brz1 = np.concatenate([b04[0, c] + bh3[0, c], b04[1, c] + bh3[1, c]])
        c1n = b04[2, c]
        bh = bh3[:, c].reshape(GR)
        in_maps.append({
            "zT": zT,
            "w0T_s": np.ascontiguousarray(w0s.T).astype(wdt),
            "wcT_s": np.ascontiguousarray(wc.T).astype(wdt),
            "lin_wT_s": np.ascontiguousarray(lin_wT[:, c * ZS:(c + 1) * ZS] * SW).astype(wdt),
            "lin_b_s": np.ascontiguousarray(lin_b[c * ZS:(c + 1) * ZS].reshape(ZS, 1)),
            "bh_s": np.ascontiguousarray(bh),
            "brz_s": np.ascontiguousarray(brz),
            "cin_s": np.ascontiguousarray(cin),
            "brz1_s": np.ascontiguousarray(brz1),
            "c1n_s": np.ascontiguousarray(c1n),
        })

    if _NC_CACHE[0] is None:
        _NC_CACHE[0] = build_nc()
    nc = _NC_CACHE[0]

    trace = os.environ.get("BASS_KERNEL_TRACE") == "1"
    if trace:
        _install_ntff_hook()
    res = run_bass_kernel_spmd(nc, in_maps, core_ids=list(range(NC)), trace=trace)
    LAST_EXEC_NS[0] = res.exec_time_ns
    LAST_RESULTS[0] = res

    full = np.empty((T, B, Z), dtype=np.float32)
    for c in range(NC):
        full[:, :, c * ZS:(c + 1) * ZS] = res.results[c]["y_part"]
    return full.transpose(1, 0, 2).reshape(B * T, Z)
